# revision 15
# baseline (speedup 1.0000x reference)
"""DetNet nms_detection kernel for 8 TRN2 NeuronCores.

Sharding: data-parallel over (batch, x-slab): core c handles batch c//4,
x-planes (c%4)*12..+12 of the 48^3 grid. The Conv3d head (the dominant
compute: 2.67G MACs) runs on the PE as 27 accumulated tap-matmuls per
output tile in fp32. Objectness sigmoid runs on the ACT engine.
The tiny proposal tail (top-128 of presigmoid logits, greedy NMS on 128
boxes, 3D IoU vs 10 GT boxes) and the scalar loss reduction are computed
on host from the gathered per-core conv outputs.
"""

import numpy as np

import concourse.bacc as bacc
import concourse.bass as bass
import concourse.mybir as mybir
from concourse import tile
from concourse.bass_utils import run_bass_kernel_spmd

F32 = mybir.dt.float32

B, N, C, X, Y, Z = 2, 10, 64, 48, 48, 48
XL = 12            # x-planes per core
YZ = Y * Z         # 2304
LOC = XL * YZ      # 27648 cells per core
PADYZ = 50 * 50    # padded yz plane
CHUNK = 384        # matmul moving size (8 y-rows of 48)
NCH = YZ // CHUNK  # 6

ANCHOR = 12.0
EPS = 1e-6
P_THRES = 0.9
IOU_THRESH = 0.2
K_PROP = 128

_CACHED = {}


def _build_program(use_f32r):
    nc = bacc.Bacc("TRN2", target_bir_lowering=False, debug=False,
                   enable_asserts=False, num_devices=8)
    fpad = nc.dram_tensor("fpad", [C, 14 * PADYZ], F32, kind="ExternalInput").ap()
    wtp = nc.dram_tensor("wtp", [128, 9 * 7], F32, kind="ExternalInput").ap()
    wtq = nc.dram_tensor("wtq", [128, 3 * 7], F32, kind="ExternalInput").ap()
    wtr = nc.dram_tensor("wtr", [C, 3 * 7], F32, kind="ExternalInput").ap()
    bia = nc.dram_tensor("bia", [7, 1], F32, kind="ExternalInput").ap()
    pred_o = nc.dram_tensor("pred", [7, LOC], F32, kind="ExternalOutput").ap()
    pobj_o = nc.dram_tensor("pobj", [128, LOC // 128], F32, kind="ExternalOutput").ap()

    QPL = 4 * PADYZ  # 4 padded x-planes per 2-xl section = 10000

    with tile.TileContext(nc) as tc:
        with tc.tile_pool(name="cst", bufs=1) as cst, \
             tc.tile_pool(name="io", bufs=2) as io, \
             tc.tile_pool(name="wk", bufs=4) as wk, \
             tc.tile_pool(name="ps", bufs=8, space="PSUM") as pp:
            wtp_sb = cst.tile([128, 9 * 7], F32)
            nc.sync.dma_start(wtp_sb[:], wtp)
            wtq_sb = cst.tile([128, 3 * 7], F32)
            nc.sync.dma_start(wtq_sb[:], wtq)
            wtr_sb = cst.tile([C, 3 * 7], F32)
            nc.sync.dma_start(wtr_sb[:], wtr)
            b_sb = cst.tile([7, 1], F32)
            nc.sync.dma_start(b_sb[:], bia)
            wpv = wtp_sb[:].rearrange("p (t o) -> p t o", t=9)
            wqv = wtq_sb[:].rearrange("p (t o) -> p t o", t=3)
            wrv = wtr_sb[:].rearrange("p (t o) -> p t o", t=3)

            for sec in range(6):
                x0 = sec * 2 * PADYZ
                # columns [0,QPL): partitions 0-63 = padded slab,
                #   64-127 = slab shifted by one z (dz pairs)
                # columns [QPL,2*QPL): partitions 0-63 = slab again,
                #   64-127 = slab shifted by one y-row (dy pairs for dz=2)
                fcomb = io.tile([128, 2 * QPL], F32, tag="fcomb")
                nc.sync.dma_start(fcomb[0:C, 0:QPL], fpad[:, x0:x0 + QPL])
                nc.sync.dma_start(fcomb[C:128, 0:QPL - 1],
                                  fpad[:, x0 + 1:x0 + QPL])
                nc.sync.dma_start(fcomb[0:C, QPL:2 * QPL],
                                  fpad[:, x0:x0 + QPL])
                nc.sync.dma_start(fcomb[C:128, QPL:2 * QPL - 50],
                                  fpad[:, x0 + 50:x0 + QPL])
                fv = fcomb[:, 0:QPL].rearrange("p (x y z) -> p x y z",
                                               x=4, y=50, z=50)
                gv = fcomb[:, QPL:2 * QPL].rearrange("p (x y z) -> p x y z",
                                                     x=4, y=50, z=50)

                for xh in range(2):
                    xl = sec * 2 + xh
                    out_sb = wk.tile([7, YZ], F32, tag="outx")
                    for ch in range(NCH):
                        ps = pp.tile([7, CHUNK], F32)
                        y0 = ch * 8
                        t = 0
                        for dx in range(3):
                            for dy in range(3):
                                # (dz=0, dz=1) fused on 128 partitions
                                rhs = fv[:, xh + dx, dy + y0:dy + y0 + 8, 0:48]
                                nc.tensor.matmul(ps[:], wpv[:, t, :], rhs,
                                                 start=(t == 0), stop=False)
                                t += 1
                        for dx in range(3):
                            # dz=2: (dy=0, dy=1) fused via y-shifted region
                            rhs = gv[:, xh + dx, y0:y0 + 8, 2:50]
                            nc.tensor.matmul(ps[:], wqv[:, dx, :], rhs,
                                             start=False, stop=False)
                            # dz=2, dy=2 single on lower 64 partitions
                            rhs2 = fv[0:C, xh + dx, y0 + 2:y0 + 10, 2:50]
                            nc.tensor.matmul(ps[:], wrv[:, dx, :], rhs2,
                                             start=False, stop=(dx == 2))
                        nc.scalar.activation(
                            out_sb[:, ch * CHUNK:(ch + 1) * CHUNK],
                            ps[:], mybir.ActivationFunctionType.Identity,
                            bias=b_sb[:], scale=1.0)
                    nc.sync.dma_start(pred_o[:, xl * YZ:(xl + 1) * YZ], out_sb[:])

            # objectness sigmoid on a [128, 216] relayout of channel 0,
            # bounced through the pred DRAM tensor
            sig_in = wk.tile([128, LOC // 128], F32)
            nc.sync.dma_start(
                sig_in[:],
                pred_o[0:1, :].rearrange("o (p f) -> (o p) f", p=128))
            sig_out = wk.tile([128, LOC // 128], F32)
            nc.scalar.activation(sig_out[:], sig_in[:],
                                 mybir.ActivationFunctionType.Sigmoid)
            nc.sync.dma_start(pobj_o, sig_out[:])

    nc.compile()
    return nc


def _get_program():
    if "nc" not in _CACHED:
        # float32r is reduced-precision (requires pre-rounded operands);
        # the top-k selection boundary needs exact fp32, so stay on fp32.
        _CACHED["nc"] = _build_program(use_f32r=False)
    return _CACHED["nc"]


# ---------------- host-side tail (exact reference semantics) ----------------

def _sigmoid_xla_f32(l):
    """XLA CPU jax.nn.sigmoid bit-semantics: s = 1f/(1f + round(exp(-x)))."""
    e = np.exp(-l.astype(np.float64)).astype(np.float32)
    t = (np.float64(1.0) + e.astype(np.float64)).astype(np.float32)
    return (np.float64(1.0) / t.astype(np.float64)).astype(np.float32)


def _iou2d(b):
    lo1 = np.maximum(b[:, None, 0], b[None, :, 0])
    lo2 = np.maximum(b[:, None, 1], b[None, :, 1])
    hi1 = np.minimum(b[:, None, 2], b[None, :, 2])
    hi2 = np.minimum(b[:, None, 3], b[None, :, 3])
    inter = np.clip(hi1 - lo1, 0.0, None) * np.clip(hi2 - lo2, 0.0, None)
    area = (b[:, 2] - b[:, 0]) * (b[:, 3] - b[:, 1])
    return inter / (area[:, None] + area[None, :] - inter + 1e-9)


def _greedy_nms(boxes, valid, thresh):
    K = boxes.shape[0]
    iou = _iou2d(boxes)
    rng = np.arange(K)
    keep = np.zeros(K, bool)
    for i in range(K):
        keep[i] = valid[i] & ~np.any(keep & (iou[i] > thresh) & (rng < i))
    return keep


def kernel(lrtlist_g, scores_g, feat_zyx, W, b):
    lrtlist_g = np.asarray(lrtlist_g, np.float32)
    scores_g = np.asarray(scores_g, np.float32)
    feat_zyx = np.asarray(feat_zyx, np.float32)
    W = np.asarray(W, np.float32)
    b = np.asarray(b, np.float32)

    nc = _get_program()

    # feat permuted zyx -> xyz, then SAME-pad all spatial dims by 1
    feat = np.transpose(feat_zyx, (0, 1, 4, 3, 2))          # [B,C,X,Y,Z]
    fp = np.pad(feat, ((0, 0), (0, 0), (1, 1), (1, 1), (1, 1)))  # [B,C,50,50,50]
    # weights: (dz=0,1) pairs on 128 partitions; for dz=2, (dy=0,1) pairs on
    # 128 partitions (y-shifted feature region) and dy=2 singles on 64
    wct = W.transpose(1, 2, 3, 4, 0)  # [C,dx,dy,dz,O]
    wtp = np.ascontiguousarray(np.concatenate([
        wct[:, :, :, 0, :].reshape(C, 9 * 7),
        wct[:, :, :, 1, :].reshape(C, 9 * 7)], axis=0))   # [128, 63]
    wtq = np.ascontiguousarray(np.concatenate([
        wct[:, :, 0, 2, :].reshape(C, 3 * 7),
        wct[:, :, 1, 2, :].reshape(C, 3 * 7)], axis=0))   # [128, 21]
    wtr = np.ascontiguousarray(wct[:, :, 2, 2, :].reshape(C, 3 * 7))
    bia = np.ascontiguousarray(b.reshape(7, 1))

    in_maps = []
    for c in range(8):
        bb, xs = c // 4, (c % 4) * XL
        slab = np.ascontiguousarray(
            fp[bb, :, xs:xs + 14, :, :].reshape(C, 14 * PADYZ))
        in_maps.append({"fpad": slab, "wtp": wtp, "wtq": wtq,
                        "wtr": wtr, "bia": bia})
    _CACHED["in_maps"] = in_maps

    res = run_bass_kernel_spmd(
        nc, in_maps, core_ids=list(range(8)), trace=False).results

    # gather
    pred = np.zeros((B, X, Y, Z, 7), np.float32)
    pred_obj = np.zeros((B, X, Y, Z), np.float32)
    for c in range(8):
        bb, xs = c // 4, (c % 4) * XL
        pr = res[c]["pred"].reshape(7, XL, Y, Z)
        pred[bb, xs:xs + XL] = np.transpose(pr, (1, 2, 3, 0))
        pred_obj[bb, xs:xs + XL] = res[c]["pobj"].reshape(XL, Y, Z)

    logits = pred[..., 0]
    pred_deltas = pred[..., 1:]

    # ---- GT decode ----
    lens = lrtlist_g[..., :3]
    rt = lrtlist_g[..., 3:].reshape(B, N, 4, 4)
    R, t = rt[..., :3, :3], rt[..., :3, 3]
    signs = np.stack(np.meshgrid([-0.5, 0.5], [-0.5, 0.5], [-0.5, 0.5],
                                 indexing='ij'), -1).reshape(8, 3).astype(np.float32)
    corners = np.einsum('bnij,bnkj->bnki', R,
                        signs[None, None] * lens[:, :, None, :]) + t[:, :, None, :]
    cmm = np.stack([corners.min(2), corners.max(2)], axis=-1)
    centers_g = t

    grid = np.stack(np.meshgrid(np.arange(X), np.arange(Y), np.arange(Z),
                                indexing='ij'), -1).astype(np.float32)
    dpos_raw = centers_g[:, :, None, None, None, :] - grid[None, None]
    dlen = np.log(lens / ANCHOR)[:, :, None, None, None, :]

    valid = scores_g[:, :, None, None, None]
    obj_dist = np.max(np.abs(dpos_raw) / (lens[:, :, None, None, None, :] * 0.5 + 1e-5), axis=-1)
    mask_pos = (obj_dist < 0.5).astype(np.float32) * valid
    mask_neg = (obj_dist < 0.8).astype(np.float32) * valid

    prev_or = ((np.cumsum(mask_pos, axis=1) - mask_pos) >= 0.5).astype(np.float32)
    contrib = mask_pos * (1.0 - prev_or)
    delta_gt = np.concatenate([dpos_raw / ANCHOR,
                               np.broadcast_to(dlen, dpos_raw.shape)], -1)
    anchor_deltas_gt = np.einsum('bnxyz,bnxyzc->bxyzc', contrib, delta_gt,
                                 optimize=True).astype(np.float32)

    pos_equal_one = (np.sum(mask_pos, axis=1) >= 0.5).astype(np.float32)
    neg_equal_one = 1.0 - (np.sum(mask_neg, axis=1) >= 0.5).astype(np.float32)
    pos_sum_safe = np.maximum(np.sum(pos_equal_one, axis=(1, 2, 3)), 1.0)

    bce = np.maximum(logits, 0.0) - logits * pos_equal_one + np.log1p(np.exp(-np.abs(logits)))
    cls_pos = np.sum(bce * pos_equal_one) / (np.sum(pos_equal_one) + EPS)
    cls_neg = np.sum(bce * neg_equal_one) / (np.sum(neg_equal_one) + EPS)
    loss_prob = 1.5 * cls_pos + 1.0 * cls_neg

    pm = pos_equal_one[..., None]
    d = pm * pred_deltas - pm * anchor_deltas_gt
    sigma2 = 9.0
    sl1 = np.where(np.abs(d) < 1.0 / sigma2, d * d * 0.5 * sigma2,
                   np.abs(d) - 0.5 / sigma2)
    loss_reg = np.sum(sl1 / pos_sum_safe[:, None, None, None, None]) / B
    total_loss = np.float32(1.0 * loss_prob + 1.0 * loss_reg)

    # ---- proposals ----
    flat_s = _sigmoid_xla_f32(logits.reshape(B, -1))
    nflat = flat_s.shape[1]
    keeps, boxes_cos, sels, ovls = [], [], [], []
    for bb in range(B):
        idx = np.lexsort((np.arange(nflat), -flat_s[bb]))[:K_PROP]
        vals = flat_s[bb][idx]
        valid_p = vals > P_THRES
        xi, yi, zi = idx // YZ, (idx // Z) % Y, idx % Z
        grid_sel = np.stack([xi, yi, zi], -1).astype(np.float32)
        d_sel = pred_deltas.reshape(B, -1, 6)[bb][idx]
        center = grid_sel + d_sel[:, :3] * ANCHOR
        half = (0.5 * np.exp(d_sel[:, 3:].astype(np.float64)) * ANCHOR).astype(np.float32)
        bmin, bmax = center - half, center + half
        boxes3d = np.stack([bmin, bmax], -1)
        vm = valid_p[:, None]
        bmin_s = np.where(vm, bmin, 0.0)
        bmax_s = np.where(vm, bmax, 0.0)
        bxy = np.concatenate([bmin_s[:, 1:], bmax_s[:, 1:]], -1)
        bzx = np.concatenate([bmin_s[:, ::2], bmax_s[:, ::2]], -1)
        keep = _greedy_nms(bxy, valid_p, IOU_THRESH) | _greedy_nms(bzx, valid_p, IOU_THRESH)
        boxes_co = np.concatenate([center, bmax - bmin], -1) * keep[:, None]
        sel = vals * keep
        lo = np.maximum(boxes3d[:, None, :, 0], cmm[bb][None, :, :, 0])
        hi = np.minimum(boxes3d[:, None, :, 1], cmm[bb][None, :, :, 1])
        inter = np.prod(np.clip(hi - lo, 0, None), -1)
        volA = np.prod(boxes3d[..., 1] - boxes3d[..., 0], -1)
        volG = np.prod(cmm[bb][..., 1] - cmm[bb][..., 0], -1)
        iou3 = inter / (volA[:, None] + volG[None, :] - inter + 1e-9)
        ovl = iou3 * keep[:, None] * (scores_g[bb] > 0)[None, :]
        keeps.append(keep)
        boxes_cos.append(boxes_co)
        sels.append(sel)
        ovls.append(ovl)

    return (total_loss,
            pred_obj.astype(np.float32),
            np.stack(boxes_cos).astype(np.float32),
            np.stack(sels).astype(np.float32),
            np.stack(keeps).astype(np.float32),
            np.stack(ovls).astype(np.float32))
